# revision 1
# baseline (speedup 1.0000x reference)
"""GroupedPNMLP forward on 8 Trainium2 NeuronCores (pure data parallel).

Per-node 2-layer MLP (32->32->1), 24 nodes in 6 groups of 4, with a
group-validity mask and node permutation.  Full inputs in, full output out;
samples are sharded N/8 per core, tiny weights replicated.

Per-core pipeline (chunks of C=2048 samples):
  DMA h in two half-chunks (natural [s, n*c] layout)
  -> ReLU in place (split GpSimd/ACT)
  -> DVE 32x32 block-transpose (channels onto partitions)
  -> mm1: 16 concurrent 32x32 TensorE sub-tile matmuls (tile_position);
     per-node weights replicated across the 4 row blocks; 6 waves of 4 nodes;
     each (row i, col jj) tile writes its own full PSUM bank (N=512) --
     hardware forbids two matmul writers on one (partition-range, bank).
  -> ACT: fused ReLU+b1 PSUM->SBUF copy
  -> mm2: block-structured W2 [128x32], col-position 32*i per sample block,
     accumulated over the 6 waves into one PSUM bank [128, 512]
  -> +b2, x group-valid mask (strided-AP reduces over the arithmetic-sequence
     group columns), DVE block-transpose back
  -> DMA out (natural layout)
"""

import numpy as np

import concourse.bass as bass
from concourse import bacc
import concourse.tile as tile
from concourse import mybir
from concourse.bass_utils import run_bass_kernel_spmd

F32 = mybir.dt.float32
I32 = mybir.dt.int32

GROUPING = np.array(
    [[0, 3, 6, 9], [1, 4, 7, 10], [2, 5, 8, 11],
     [12, 13, 14, 15], [16, 18, 20, 22], [17, 19, 21, 23]], dtype=np.int32)

N_CORES = 8
S_TOT = 131072
S = S_TOT // N_CORES      # 16384 samples per core
NODES = 24
CH = 32                   # in channels = hidden dim
C = 2048                  # samples per chunk
NSUB = C // 128           # 16 sub-tiles of 128 samples
HSUB = NSUB // 2          # 8 sub-tiles per half-chunk
CQ = C // 4               # 512 samples per i-block = one matmul N = one bank
NCHUNK = S // C           # 8
NW = 6                    # waves of 4 nodes


def _build_program():
    nc = bacc.Bacc(None, target_bir_lowering=False)

    h = nc.dram_tensor("h", [S, NODES * CH], F32, kind="ExternalInput")
    valid = nc.dram_tensor("valid", [S, NODES], I32, kind="ExternalInput")
    w1rep = nc.dram_tensor("w1rep", [128, NW, 4, CH], F32, kind="ExternalInput")
    w2blk = nc.dram_tensor("w2blk", [128, NW, 32], F32, kind="ExternalInput")
    b1col = nc.dram_tensor("b1col", [128, NW], F32, kind="ExternalInput")
    b2col = nc.dram_tensor("b2col", [128, 1], F32, kind="ExternalInput")
    out = nc.dram_tensor("out", [S, NODES], F32, kind="ExternalOutput")

    with tile.TileContext(nc) as tc:
        with (
            tc.tile_pool(name="singles", bufs=1) as singles,
            tc.tile_pool(name="xp", bufs=2) as xp,
            tc.tile_pool(name="xtp", bufs=2) as xtp,
            tc.tile_pool(name="hidp", bufs=2) as hidp,
            tc.tile_pool(name="vp", bufs=2) as vp,
            tc.tile_pool(name="op", bufs=2) as op,
            tc.tile_pool(name="pha_pool", bufs=2, space="PSUM") as pha_pool,
            tc.tile_pool(name="phb_pool", bufs=1, space="PSUM") as phb_pool,
            tc.tile_pool(name="p2_pool", bufs=2, space="PSUM") as p2_pool,
        ):
            w1sb = singles.tile([128, NW, 4, CH], F32)
            nc.sync.dma_start(out=w1sb, in_=w1rep[:, :, :, :])
            w2sb = singles.tile([128, NW, 32], F32)
            nc.sync.dma_start(out=w2sb, in_=w2blk[:, :, :])
            b1sb = singles.tile([128, NW], F32)
            nc.sync.dma_start(out=b1sb, in_=b1col[:, :])
            b2sb = singles.tile([128, 1], F32)
            nc.sync.dma_start(out=b2sb, in_=b2col[:, :])

            for cc in range(NCHUNK):
                c0 = cc * C
                # ---- load x in halves, relu, transpose into xt ----
                xt = xtp.tile([128, NSUB, NODES, CH], F32)
                for hh in range(2):
                    xh = xp.tile([128, HSUB, NODES, CH], F32)
                    lo = c0 + hh * (C // 2)
                    nc.sync.dma_start(
                        out=xh.rearrange("p s n c -> p s (n c)"),
                        in_=h[lo:lo + C // 2, :].rearrange(
                            "(s p) f -> p s f", p=128),
                    )
                    qq = HSUB // 2
                    nc.gpsimd.tensor_scalar_max(
                        xh[:, 0:qq].rearrange("p s n c -> p (s n c)"),
                        xh[:, 0:qq].rearrange("p s n c -> p (s n c)"), 0.0)
                    nc.scalar.activation(
                        xh[:, qq:HSUB].rearrange("p s n c -> p (s n c)"),
                        xh[:, qq:HSUB].rearrange("p s n c -> p (s n c)"),
                        mybir.ActivationFunctionType.Relu)
                    nc.vector.transpose(
                        xt[:, hh * HSUB:(hh + 1) * HSUB], xh)

                # ---- valid -> group mask (natural layout) ----
                vi = vp.tile([128, NSUB, NODES], I32)
                nc.scalar.dma_start(
                    out=vi,
                    in_=valid[c0:c0 + C, :].rearrange("(s p) n -> p s n", p=128),
                )
                vf = vp.tile([128, NSUB, NODES], F32)
                nc.gpsimd.tensor_copy(vf, vi)
                gv = vp.tile([128, NSUB, 8], F32)
                nc.vector.tensor_reduce(
                    gv[:, :, 0:3],
                    vf[:, :, 0:12].rearrange("p s (k g) -> p s g k", g=3),
                    axis=mybir.AxisListType.X, op=mybir.AluOpType.add)
                nc.vector.tensor_reduce(
                    gv[:, :, 3:4], vf[:, :, 12:16],
                    axis=mybir.AxisListType.X, op=mybir.AluOpType.add)
                nc.vector.tensor_reduce(
                    gv[:, :, 4:6],
                    vf[:, :, 16:24].rearrange("p s (k g) -> p s g k", g=2),
                    axis=mybir.AxisListType.X, op=mybir.AluOpType.add)
                nc.gpsimd.tensor_scalar(
                    gv[:, :, 0:6], gv[:, :, 0:6], 0.5, None,
                    op0=mybir.AluOpType.is_gt)
                mask_nat = vp.tile([128, NSUB, 32], F32)
                nc.gpsimd.memset(mask_nat[:, :, 24:32], 0.0)
                nc.gpsimd.tensor_copy(
                    mask_nat[:, :, 0:12].rearrange("p s (k g) -> p s g k", g=3),
                    gv[:, :, 0:3].broadcast_to([128, NSUB, 3, 4]))
                nc.gpsimd.tensor_copy(
                    mask_nat[:, :, 12:16],
                    gv[:, :, 3:4].broadcast_to([128, NSUB, 4]))
                nc.gpsimd.tensor_copy(
                    mask_nat[:, :, 16:24].rearrange("p s (k g) -> p s g k", g=2),
                    gv[:, :, 4:6].broadcast_to([128, NSUB, 2, 4]))
                maskT = vp.tile([128, NSUB, 32], F32)
                nc.vector.transpose(maskT, mask_nat)

                # ---- the 2-layer MLP on TensorE ----
                p2 = p2_pool.tile([128, NSUB, 32], F32)  # [(i,n), (sub,si)]
                p2f = p2.rearrange("p a b -> p (a b)")
                for w in range(NW):
                    pha = pha_pool.tile([128, 2 * CQ], F32)  # i = 0, 1
                    phb = phb_pool.tile([128, 2 * CQ], F32)  # i = 2, 3
                    for i in range(4):
                        ph_t = pha if i < 2 else phb
                        fo = (i % 2) * CQ
                        for jj in range(4):
                            n = 4 * w + jj
                            nc.tensor.matmul(
                                ph_t[32 * jj:32 * jj + 32, fo:fo + CQ],
                                lhsT=w1sb[32 * i:32 * i + 32, w, jj, :],
                                rhs=xt[32 * i:32 * i + 32, :, n, :],
                                start=True, stop=True,
                                tile_position=(32 * i, 32 * jj))
                    hid = hidp.tile([128, C], F32)
                    nc.scalar.activation(
                        hid[:, 0:2 * CQ], pha,
                        mybir.ActivationFunctionType.Relu,
                        bias=b1sb[:, w:w + 1])
                    nc.scalar.activation(
                        hid[:, 2 * CQ:4 * CQ], phb,
                        mybir.ActivationFunctionType.Relu,
                        bias=b1sb[:, w:w + 1])
                    for i in range(4):
                        nc.tensor.matmul(
                            p2f[32 * i:32 * i + 32, :],
                            lhsT=w2sb[:, w, :],
                            rhs=hid[:, i * CQ:(i + 1) * CQ],
                            start=(w == 0), stop=(w == NW - 1),
                            skip_group_check=True,
                            tile_position=(0, 32 * i))

                # ---- +b2, mask, transpose back, store ----
                m2 = op.tile([128, NSUB, 32], F32)
                nc.vector.tensor_scalar(
                    m2, p2, b2sb[:, 0:1], None, op0=mybir.AluOpType.add)
                nc.vector.tensor_tensor(
                    m2, m2, maskT, op=mybir.AluOpType.mult)
                outT = op.tile([128, NSUB, 32], F32)
                nc.vector.transpose(outT, m2)
                nc.scalar.dma_start(
                    out=out[c0:c0 + C, :].rearrange("(s p) n -> p s n", p=128),
                    in_=outT[:, :, 0:NODES],
                )

    nc.compile()
    return nc


_PROGRAM = None


def _get_program():
    global _PROGRAM
    if _PROGRAM is None:
        _PROGRAM = _build_program()
    return _PROGRAM


def _prep_weights(W1, b1, W2, b2):
    flat = GROUPING.reshape(-1)
    g_of = np.zeros(NODES, np.int64)
    k_of = np.zeros(NODES, np.int64)
    for q, nid in enumerate(flat):
        g_of[nid] = q // 4
        k_of[nid] = q % 4
    W1n = np.ascontiguousarray(W1[g_of, k_of]).astype(np.float32)  # [24,32,32]
    W2n = np.ascontiguousarray(W2[g_of, k_of]).astype(np.float32)  # [24,32,1]
    b1n = np.ascontiguousarray(b1[g_of, k_of]).astype(np.float32)  # [24,32]
    b2n = np.ascontiguousarray(b2[g_of, k_of]).astype(np.float32)  # [24,1]

    w1rep = np.zeros((128, NW, 4, CH), np.float32)
    w2blk = np.zeros((128, NW, 32), np.float32)
    b1col = np.zeros((128, NW), np.float32)
    b2col = np.zeros((128, 1), np.float32)
    for w in range(NW):
        for jj in range(4):
            n = 4 * w + jj
            for r in range(4):
                w1rep[32 * r:32 * r + 32, w, jj, :] = W1n[n]
            w2blk[32 * jj:32 * jj + 32, w, n] = W2n[n][:, 0]
            b1col[32 * jj:32 * jj + 32, w] = b1n[n]
    for i in range(4):
        b2col[32 * i:32 * i + 24, 0] = b2n[:, 0]
    return w1rep, w2blk, b1col, b2col


def kernel(h, valid, W1, b1, W2, b2):
    nc = _get_program()
    w1rep, w2blk, b1col, b2col = _prep_weights(W1, b1, W2, b2)
    h2 = np.ascontiguousarray(h, dtype=np.float32).reshape(S_TOT, NODES * CH)
    v2 = np.ascontiguousarray(valid, dtype=np.int32).reshape(S_TOT, NODES)

    in_maps = []
    for c in range(N_CORES):
        sl = slice(c * S, (c + 1) * S)
        in_maps.append({
            "h": h2[sl],
            "valid": v2[sl],
            "w1rep": w1rep,
            "w2blk": w2blk,
            "b1col": b1col,
            "b2col": b2col,
        })
    res = run_bass_kernel_spmd(nc, in_maps, core_ids=list(range(N_CORES)))
    outs = [res.results[c]["out"] for c in range(N_CORES)]
    full = np.concatenate(outs, axis=0).astype(np.float32)
    return full.reshape(S_TOT, NODES, 1)



# revision 7
# speedup vs baseline: 3.5007x; 3.5007x over previous
"""GroupedPNMLP forward on 8 Trainium2 NeuronCores (pure data parallel).

Per-node 2-layer MLP (32->32->1), 24 nodes in 6 groups of 4, with a
group-validity mask and node permutation.  Full inputs in, full output out;
samples are sharded N/8 per core, tiny weights replicated.

v2 pipeline (chunks of C=2048 samples, block-cyclic sample->partition):
  DMA h in two half-chunks (24KB contiguous per partition)
  -> ReLU + downcast to bf16 on ScalarE (one ACTIVATE per half)
  -> DVE 32x32 block-transpose (channels onto partitions, bf16)
  -> mm1: ONE 128x128 block-diagonal bf16 matmul per node (4 copies of
     W1n on the diagonal -> all 4 sample sub-blocks in one shot); free
     dim 512 = one PSUM bank per node
  -> hidden drain: ReLU + b1, PSUM->SBUF bf16; alternating ScalarE /
     DVE per node so both engines drain concurrently
  -> mm2: 128x128 block-diagonal W2 (column slot = node id), 24 bf16
     matmuls accumulating into one PSUM bank [128, 512]
  -> +b2 on ScalarE, DVE transpose back, group-valid mask multiply in
     natural layout, DMA out
  Mask path (small) runs on GpSimd + DVE off the critical path.
"""

import numpy as np
import ml_dtypes

import concourse.bass as bass
from concourse import bacc
import concourse.tile as tile
from concourse import mybir
from concourse.bass_utils import run_bass_kernel_spmd

F32 = mybir.dt.float32
BF16 = mybir.dt.bfloat16
I32 = mybir.dt.int32

GROUPING = np.array(
    [[0, 3, 6, 9], [1, 4, 7, 10], [2, 5, 8, 11],
     [12, 13, 14, 15], [16, 18, 20, 22], [17, 19, 21, 23]], dtype=np.int32)

N_CORES = 8
S_TOT = 131072
S = S_TOT // N_CORES      # 16384 samples per core
NODES = 24
CH = 32                   # in channels = hidden dim
C = 2048                  # samples per chunk
NSUB = C // 128           # 16 blocks of 32 samples (pi) per chunk
HSUB = NSUB // 2          # 8 per half-chunk
NCHUNK = S // C           # 8
NW = 6                    # waves of 4 nodes

SA = 4                    # input-relu sub-tiles (of HSUB) on ScalarE; rest DVE
# drain engine per (wave, half): 'A' = ScalarE ACTIVATE, 'V' = DVE
DRAIN_PATTERN = "AVAAVAAVAAVA"


def _build_program():
    nc = bacc.Bacc(None, target_bir_lowering=False)

    h = nc.dram_tensor("h", [S, NODES * CH], F32, kind="ExternalInput")
    valid = nc.dram_tensor("valid", [S, NODES], I32, kind="ExternalInput")
    w1d = nc.dram_tensor("w1d", [128, NODES, 128], BF16, kind="ExternalInput")
    w2d = nc.dram_tensor("w2d", [128, NODES, 128], BF16, kind="ExternalInput")
    b1c = nc.dram_tensor("b1c", [128, NODES], F32, kind="ExternalInput")
    b2c = nc.dram_tensor("b2c", [128, 1], F32, kind="ExternalInput")
    out = nc.dram_tensor("out", [S, NODES], F32, kind="ExternalOutput")

    with tile.TileContext(nc) as tc:
        with (
            tc.tile_pool(name="singles", bufs=1) as singles,
            tc.tile_pool(name="xp", bufs=2) as xp,
            tc.tile_pool(name="xrp", bufs=2) as xrp,
            tc.tile_pool(name="xtp", bufs=2) as xtp,
            tc.tile_pool(name="hp", bufs=3) as hp,
            tc.tile_pool(name="vp", bufs=2) as vp,
            tc.tile_pool(name="op", bufs=2) as op,
            tc.tile_pool(name="php", bufs=3, space="PSUM") as php,
            tc.tile_pool(name="p2p", bufs=2, space="PSUM") as p2p,
        ):
            w1sb = singles.tile([128, NODES, 128], BF16)
            nc.sync.dma_start(out=w1sb, in_=w1d[:, :, :])
            w2sb = singles.tile([128, NODES, 128], BF16)
            nc.sync.dma_start(out=w2sb, in_=w2d[:, :, :])
            b1sb = singles.tile([128, NODES], F32)
            nc.sync.dma_start(out=b1sb, in_=b1c[:, :])
            b2sb = singles.tile([128, 1], F32)
            nc.sync.dma_start(out=b2sb, in_=b2c[:, :])

            for cc in range(NCHUNK):
                c0 = cc * C
                # ---- load h in halves, relu+cast bf16, transpose ----
                xt = xtp.tile([128, NSUB, NODES, CH], BF16)
                for hh in range(2):
                    xh = xp.tile([128, HSUB, NODES, CH], F32)
                    lo = c0 + hh * (C // 2)
                    nc.sync.dma_start(
                        out=xh.rearrange("p s n c -> p (s n c)"),
                        in_=h[lo:lo + C // 2, :].rearrange(
                            "(p s) f -> p (s f)", p=128),
                    )
                    xr = xrp.tile([128, HSUB, NODES, CH], BF16)
                    if SA > 0:
                        nc.scalar.activation(
                            xr[:, 0:SA].rearrange("p s n c -> p (s n c)"),
                            xh[:, 0:SA].rearrange("p s n c -> p (s n c)"),
                            mybir.ActivationFunctionType.Relu)
                    if SA < HSUB:
                        nc.vector.tensor_scalar_max(
                            xr[:, SA:HSUB].rearrange("p s n c -> p (s n c)"),
                            xh[:, SA:HSUB].rearrange("p s n c -> p (s n c)"),
                            0.0)
                    nc.vector.transpose(
                        xt[:, hh * HSUB:(hh + 1) * HSUB], xr)

                # ---- valid -> group mask (natural layout, off crit path) ----
                vi = vp.tile([128, NSUB, NODES], I32)
                for hh in range(2):
                    lo = c0 + hh * (C // 2)
                    nc.gpsimd.dma_start(
                        out=vi[:, hh * HSUB:(hh + 1) * HSUB],
                        in_=valid[lo:lo + C // 2, :].rearrange(
                            "(p s) n -> p s n", p=128),
                    )
                vf = vp.tile([128, NSUB, NODES], F32)
                nc.gpsimd.tensor_copy(vf, vi)
                gv = vp.tile([128, NSUB, 8], F32)
                nc.vector.tensor_reduce(
                    gv[:, :, 0:3],
                    vf[:, :, 0:12].rearrange("p s (k g) -> p s g k", g=3),
                    axis=mybir.AxisListType.X, op=mybir.AluOpType.add)
                nc.vector.tensor_reduce(
                    gv[:, :, 3:4], vf[:, :, 12:16],
                    axis=mybir.AxisListType.X, op=mybir.AluOpType.add)
                nc.vector.tensor_reduce(
                    gv[:, :, 4:6],
                    vf[:, :, 16:24].rearrange("p s (k g) -> p s g k", g=2),
                    axis=mybir.AxisListType.X, op=mybir.AluOpType.add)
                nc.vector.tensor_scalar(
                    gv[:, :, 0:6], gv[:, :, 0:6], 0.5, None,
                    op0=mybir.AluOpType.is_gt)
                mk = vp.tile([128, NSUB, NODES], F32)
                nc.gpsimd.tensor_copy(
                    mk[:, :, 0:12].rearrange("p s (k g) -> p s g k", g=3),
                    gv[:, :, 0:3].broadcast_to([128, NSUB, 3, 4]))
                nc.gpsimd.tensor_copy(
                    mk[:, :, 12:16],
                    gv[:, :, 3:4].broadcast_to([128, NSUB, 4]))
                nc.gpsimd.tensor_copy(
                    mk[:, :, 16:24].rearrange("p s (k g) -> p s g k", g=2),
                    gv[:, :, 4:6].broadcast_to([128, NSUB, 2, 4]))

                # ---- the 2-layer MLP on TensorE (block-diag 128x128) ----
                p2 = p2p.tile([128, NSUB, CH], F32)
                p2f = p2.rearrange("p a b -> p (a b)")
                for w in range(NW):
                    for half in range(2):
                        ph = php.tile([128, 2, 512], F32)
                        hidt = hp.tile([128, 2, 512], BF16)
                        for q in range(2):
                            n = 4 * w + half * 2 + q
                            nc.tensor.matmul(
                                ph[:, q, :],
                                lhsT=w1sb[:, n, :],
                                rhs=xt[:, :, n, :],
                                start=True, stop=True)
                        n0 = 4 * w + half * 2
                        if DRAIN_PATTERN[2 * w + half] == "A":
                            nc.scalar.activation(
                                hidt.rearrange("p a b -> p (a b)"),
                                ph.rearrange("p a b -> p (a b)"),
                                mybir.ActivationFunctionType.Relu,
                                bias=b1sb[:, n0:n0 + 1])
                        else:
                            nc.vector.tensor_scalar(
                                hidt.rearrange("p a b -> p (a b)"),
                                ph.rearrange("p a b -> p (a b)"),
                                b1sb[:, n0:n0 + 1], 0.0,
                                op0=mybir.AluOpType.add,
                                op1=mybir.AluOpType.max)
                        for q in range(2):
                            n = 4 * w + half * 2 + q
                            nc.tensor.matmul(
                                p2f,
                                lhsT=w2sb[:, n, :],
                                rhs=hidt[:, q, :],
                                start=(n == 0), stop=(n == NODES - 1),
                                skip_group_check=True)

                # ---- +b2, transpose back, mask, store ----
                m2 = op.tile([128, NSUB, CH], F32)
                nc.scalar.activation(
                    m2.rearrange("p a b -> p (a b)"), p2f,
                    mybir.ActivationFunctionType.Identity,
                    bias=b2sb[:, 0:1])
                outT = op.tile([128, NSUB, CH], F32)
                nc.vector.transpose(outT, m2)
                outF = op.tile([128, NSUB, NODES], F32)
                nc.vector.tensor_tensor(
                    outF, outT[:, :, 0:NODES], mk, op=mybir.AluOpType.mult)
                for hh in range(2):
                    lo = c0 + hh * (C // 2)
                    nc.gpsimd.dma_start(
                        out=out[lo:lo + C // 2, :].rearrange(
                            "(p s) n -> p s n", p=128),
                        in_=outF[:, hh * HSUB:(hh + 1) * HSUB],
                    )

    nc.compile()
    return nc


_PROGRAM = None


def _get_program():
    global _PROGRAM
    if _PROGRAM is None:
        _PROGRAM = _build_program()
    return _PROGRAM


def _prep_weights(W1, b1, W2, b2):
    flat = GROUPING.reshape(-1)
    g_of = np.zeros(NODES, np.int64)
    k_of = np.zeros(NODES, np.int64)
    for q, nid in enumerate(flat):
        g_of[nid] = q // 4
        k_of[nid] = q % 4
    W1n = np.ascontiguousarray(W1[g_of, k_of]).astype(np.float32)  # [24,32,32]
    W2n = np.ascontiguousarray(W2[g_of, k_of]).astype(np.float32)  # [24,32,1]
    b1n = np.ascontiguousarray(b1[g_of, k_of]).astype(np.float32)  # [24,32]
    b2n = np.ascontiguousarray(b2[g_of, k_of]).astype(np.float32)  # [24,1]

    w1dv = np.zeros((128, NODES, 128), np.float32)
    w2dv = np.zeros((128, NODES, 128), np.float32)
    b1cv = np.zeros((128, NODES), np.float32)
    b2cv = np.zeros((128, 1), np.float32)
    for n in range(NODES):
        for i in range(4):
            sl = slice(32 * i, 32 * i + 32)
            w1dv[sl, n, sl] = W1n[n]                       # [c, h] block
            w2dv[sl, n, 32 * i + n] = W2n[n][:, 0]         # column slot n
            b1cv[sl, n] = b1n[n]
            b2cv[32 * i + n, 0] = b2n[n, 0]
    return (w1dv.astype(ml_dtypes.bfloat16), w2dv.astype(ml_dtypes.bfloat16),
            b1cv, b2cv)


def _make_in_maps(inputs):
    w1dv, w2dv, b1cv, b2cv = _prep_weights(
        inputs["W1"], inputs["b1"], inputs["W2"], inputs["b2"])
    h2 = np.ascontiguousarray(inputs["h"], dtype=np.float32).reshape(
        S_TOT, NODES * CH)
    v2 = np.ascontiguousarray(inputs["valid"], dtype=np.int32).reshape(
        S_TOT, NODES)
    in_maps = []
    for c in range(N_CORES):
        sl = slice(c * S, (c + 1) * S)
        in_maps.append({
            "h": h2[sl],
            "valid": v2[sl],
            "w1d": w1dv,
            "w2d": w2dv,
            "b1c": b1cv,
            "b2c": b2cv,
        })
    return in_maps


def kernel(h, valid, W1, b1, W2, b2):
    nc = _get_program()
    in_maps = _make_in_maps(
        {"h": h, "valid": valid, "W1": W1, "b1": b1, "W2": W2, "b2": b2})
    res = run_bass_kernel_spmd(nc, in_maps, core_ids=list(range(N_CORES)))
    outs = [res.results[c]["out"] for c in range(N_CORES)]
    full = np.concatenate(outs, axis=0).astype(np.float32)
    return full.reshape(S_TOT, NODES, 1)


# revision 12
# speedup vs baseline: 4.1242x; 1.1781x over previous
"""GroupedPNMLP forward on 8 Trainium2 NeuronCores (pure data parallel).

Per-node 2-layer MLP (32->32->1), 24 nodes in 6 groups of 4, with a
group-validity mask and node permutation.  Full inputs in, full output out;
samples are sharded N/8 per core, tiny weights replicated.

v4 pipeline (chunks of C=2048 samples, block-cyclic sample->partition):
  DMA h in two half-chunks (24KB contiguous per partition)
  -> ReLU + downcast to bf16 on DVE (2x_2p mode), written in a
     [n, t, c, s2] swizzle so neighbouring samples (s2) pair up
  -> DVE 32x32 block-transpose on the uint32 *pair* view: half the
     elements of an elementwise transpose; channels land on partitions
  -> mm1: ONE 128x128 block-diagonal bf16 matmul per node (4 copies of
     W1n on the diagonal -> all 4 sample sub-blocks in one shot); free
     dim 512 = one PSUM bank per node
  -> hidden drain: ReLU (+b1), PSUM->SBUF bf16 on ScalarE, one ACTIVATE
     per 2-bank pair
  -> mm2: 128x128 block-diagonal W2 (column slot = node id), 24 bf16
     matmuls accumulating into one PSUM bank; the rhs access pattern
     re-enumerates samples (hh,t,s2,pi) so pi is innermost again
  -> +b2 on ScalarE, DVE transpose back, group-valid mask multiply on
     GpSimd in natural layout, DMA out
  Mask path (small) runs on GpSimd + DVE off the critical path.
"""

import numpy as np
import ml_dtypes

import concourse.bass as bass
from concourse import bacc
import concourse.tile as tile
from concourse import mybir
from concourse.bass_utils import run_bass_kernel_spmd

F32 = mybir.dt.float32
BF16 = mybir.dt.bfloat16
I32 = mybir.dt.int32
U32 = mybir.dt.uint32

GROUPING = np.array(
    [[0, 3, 6, 9], [1, 4, 7, 10], [2, 5, 8, 11],
     [12, 13, 14, 15], [16, 18, 20, 22], [17, 19, 21, 23]], dtype=np.int32)

N_CORES = 8
S_TOT = 131072
S = S_TOT // N_CORES      # 16384 samples per core
NODES = 24
CH = 32                   # in channels = hidden dim
C = 2048                  # samples per chunk
NSUB = C // 128           # 16 samples per partition per chunk
HSUB = NSUB // 2          # 8 per half-chunk (t in 0..3, s2 in 0..1)
NCHUNK = S // C           # 8
NW = 6                    # waves of 4 nodes

# input-relu t-slices (of 4) on ScalarE; rest on DVE
ACT_T = 0


def _build_program():
    nc = bacc.Bacc(None, target_bir_lowering=False)

    h = nc.dram_tensor("h", [S, NODES * CH], F32, kind="ExternalInput")
    valid = nc.dram_tensor("valid", [S, NODES], I32, kind="ExternalInput")
    w1d = nc.dram_tensor("w1d", [128, NODES, 128], BF16, kind="ExternalInput")
    w2d = nc.dram_tensor("w2d", [128, NODES, 128], BF16, kind="ExternalInput")
    b1c = nc.dram_tensor("b1c", [128, NODES], F32, kind="ExternalInput")
    b2c = nc.dram_tensor("b2c", [128, 1], F32, kind="ExternalInput")
    out = nc.dram_tensor("out", [S, NODES], F32, kind="ExternalOutput")

    with tile.TileContext(nc) as tc:
        with (
            tc.tile_pool(name="singles", bufs=1) as singles,
            tc.tile_pool(name="xp", bufs=2) as xp,
            tc.tile_pool(name="xrp", bufs=2) as xrp,
            tc.tile_pool(name="xtp", bufs=2) as xtp,
            tc.tile_pool(name="hp", bufs=3) as hp,
            tc.tile_pool(name="vp", bufs=2) as vp,
            tc.tile_pool(name="op", bufs=2) as op,
            tc.tile_pool(name="php", bufs=3, space="PSUM") as php,
            tc.tile_pool(name="p2p", bufs=2, space="PSUM") as p2p,
        ):
            w1sb = singles.tile([128, NODES, 128], BF16)
            nc.sync.dma_start(out=w1sb, in_=w1d[:, :, :])
            w2sb = singles.tile([128, NODES, 128], BF16)
            nc.sync.dma_start(out=w2sb, in_=w2d[:, :, :])
            b1sb = singles.tile([128, NODES], F32)
            nc.sync.dma_start(out=b1sb, in_=b1c[:, :])
            b2sb = singles.tile([128, 1], F32)
            nc.sync.dma_start(out=b2sb, in_=b2c[:, :])

            for cc in range(NCHUNK):
                c0 = cc * C
                # ---- load h in halves, relu+cast bf16 swizzled, transpose ----
                # xt[32b+c, hh, n, t, 2*pi+s2] = relu(h[sample, n, c]) where
                # sample = c0 + hh*1024 + (32b+pi)*8 + 2*t + s2
                xt = xtp.tile([128, 2, NODES, 4, 64], BF16)
                for hh in range(2):
                    xh = xp.tile([128, HSUB, NODES, CH], F32)
                    lo = c0 + hh * (C // 2)
                    nc.sync.dma_start(
                        out=xh.rearrange("p s n c -> p (s n c)"),
                        in_=h[lo:lo + C // 2, :].rearrange(
                            "(p s) f -> p (s f)", p=128),
                    )
                    # xr[p, n, t, (c, s2)] = relu(xh[p, 2t+s2, n, c]) in bf16
                    # (3D-AP limit: one instruction per s2 phase)
                    xr = xrp.tile([128, NODES, 4, 64], BF16)
                    xr_v = xr.rearrange("p n t (c s2) -> p s2 t n c", s2=2)
                    xh_v = xh.rearrange("p (t s2) n c -> p s2 t n c", s2=2)
                    for s2 in range(2):
                        if ACT_T > 0:
                            nc.scalar.activation(
                                xr_v[:, s2, 0:ACT_T], xh_v[:, s2, 0:ACT_T],
                                mybir.ActivationFunctionType.Relu)
                        if ACT_T < 4:
                            nc.vector.tensor_scalar_max(
                                xr_v[:, s2, ACT_T:4], xh_v[:, s2, ACT_T:4],
                                0.0)
                    # u32 pair transpose: channels onto partitions
                    nc.vector.transpose(
                        xt[:, hh].bitcast(U32), xr.bitcast(U32))

                # ---- valid -> group mask (natural layout, off crit path) ----
                vi = vp.tile([128, NSUB, NODES], I32)
                for hh in range(2):
                    lo = c0 + hh * (C // 2)
                    nc.gpsimd.dma_start(
                        out=vi[:, hh * HSUB:(hh + 1) * HSUB],
                        in_=valid[lo:lo + C // 2, :].rearrange(
                            "(p s) n -> p s n", p=128),
                    )
                vf = vp.tile([128, NSUB, NODES], F32)
                nc.gpsimd.tensor_copy(vf, vi)
                gv = vp.tile([128, NSUB, 8], F32)
                nc.vector.tensor_reduce(
                    gv[:, :, 0:3],
                    vf[:, :, 0:12].rearrange("p s (k g) -> p s g k", g=3),
                    axis=mybir.AxisListType.X, op=mybir.AluOpType.add)
                nc.vector.tensor_reduce(
                    gv[:, :, 3:4], vf[:, :, 12:16],
                    axis=mybir.AxisListType.X, op=mybir.AluOpType.add)
                nc.vector.tensor_reduce(
                    gv[:, :, 4:6],
                    vf[:, :, 16:24].rearrange("p s (k g) -> p s g k", g=2),
                    axis=mybir.AxisListType.X, op=mybir.AluOpType.add)
                nc.vector.tensor_scalar(
                    gv[:, :, 0:6], gv[:, :, 0:6], 0.5, None,
                    op0=mybir.AluOpType.is_gt)
                mk = vp.tile([128, NSUB, NODES], F32)
                nc.gpsimd.tensor_copy(
                    mk[:, :, 0:12].rearrange("p s (k g) -> p s g k", g=3),
                    gv[:, :, 0:3].broadcast_to([128, NSUB, 3, 4]))
                nc.gpsimd.tensor_copy(
                    mk[:, :, 12:16],
                    gv[:, :, 3:4].broadcast_to([128, NSUB, 4]))
                nc.gpsimd.tensor_copy(
                    mk[:, :, 16:24].rearrange("p s (k g) -> p s g k", g=2),
                    gv[:, :, 4:6].broadcast_to([128, NSUB, 2, 4]))

                # ---- the 2-layer MLP on TensorE (block-diag 128x128) ----
                p2 = p2p.tile([128, NSUB, CH], F32)   # ((ht, s2), pi)
                p2f = p2.rearrange("p a b -> p (a b)")
                for w in range(NW):
                    for half in range(2):
                        ph = php.tile([128, 2, 512], F32)
                        hidt = hp.tile([128, 2, 512], BF16)
                        for q in range(2):
                            n = 4 * w + half * 2 + q
                            nc.tensor.matmul(
                                ph[:, q, :],
                                lhsT=w1sb[:, n, :],
                                rhs=xt[:, :, n, :, :],
                                start=True, stop=True)
                        n0 = 4 * w + half * 2
                        nc.scalar.activation(
                            hidt.rearrange("p a b -> p (a b)"),
                            ph.rearrange("p a b -> p (a b)"),
                            mybir.ActivationFunctionType.Relu,
                            bias=b1sb[:, n0:n0 + 1])
                        for q in range(2):
                            n = 4 * w + half * 2 + q
                            nc.tensor.matmul(
                                p2f,
                                lhsT=w2sb[:, n, :],
                                rhs=hidt[:, q].rearrange(
                                    "p (ht pi s2) -> p ht s2 pi",
                                    ht=8, s2=2),
                                start=(n == 0), stop=(n == NODES - 1),
                                skip_group_check=True)

                # ---- +b2, transpose back, mask, store ----
                # p2 block f = (ht, s2) enumerates sub = hh*8 + 2t + s2
                # sequentially, so outT free is natural (sub, node) order.
                m2 = op.tile([128, NSUB, CH], F32)
                nc.scalar.activation(
                    m2.rearrange("p a b -> p (a b)"), p2f,
                    mybir.ActivationFunctionType.Identity,
                    bias=b2sb[:, 0:1])
                outT = op.tile([128, NSUB, CH], F32)
                nc.vector.transpose(outT, m2)
                outF = op.tile([128, NSUB, NODES], F32)
                nc.gpsimd.tensor_tensor(
                    outF, outT[:, :, 0:NODES], mk, op=mybir.AluOpType.mult)
                for hh in range(2):
                    lo = c0 + hh * (C // 2)
                    nc.gpsimd.dma_start(
                        out=out[lo:lo + C // 2, :].rearrange(
                            "(p s) n -> p s n", p=128),
                        in_=outF[:, hh * HSUB:(hh + 1) * HSUB],
                    )

    nc.compile()
    return nc


_PROGRAM = None


def _get_program():
    global _PROGRAM
    if _PROGRAM is None:
        _PROGRAM = _build_program()
    return _PROGRAM


def _prep_weights(W1, b1, W2, b2):
    flat = GROUPING.reshape(-1)
    g_of = np.zeros(NODES, np.int64)
    k_of = np.zeros(NODES, np.int64)
    for q, nid in enumerate(flat):
        g_of[nid] = q // 4
        k_of[nid] = q % 4
    W1n = np.ascontiguousarray(W1[g_of, k_of]).astype(np.float32)  # [24,32,32]
    W2n = np.ascontiguousarray(W2[g_of, k_of]).astype(np.float32)  # [24,32,1]
    b1n = np.ascontiguousarray(b1[g_of, k_of]).astype(np.float32)  # [24,32]
    b2n = np.ascontiguousarray(b2[g_of, k_of]).astype(np.float32)  # [24,1]

    w1dv = np.zeros((128, NODES, 128), np.float32)
    w2dv = np.zeros((128, NODES, 128), np.float32)
    b1cv = np.zeros((128, NODES), np.float32)
    b2cv = np.zeros((128, 1), np.float32)
    for n in range(NODES):
        for i in range(4):
            sl = slice(32 * i, 32 * i + 32)
            w1dv[sl, n, sl] = W1n[n]                       # [c, h] block
            w2dv[sl, n, 32 * i + n] = W2n[n][:, 0]         # column slot n
            b1cv[sl, n] = b1n[n]
            b2cv[32 * i + n, 0] = b2n[n, 0]
    return (w1dv.astype(ml_dtypes.bfloat16), w2dv.astype(ml_dtypes.bfloat16),
            b1cv, b2cv)


def _make_in_maps(inputs):
    w1dv, w2dv, b1cv, b2cv = _prep_weights(
        inputs["W1"], inputs["b1"], inputs["W2"], inputs["b2"])
    h2 = np.ascontiguousarray(inputs["h"], dtype=np.float32).reshape(
        S_TOT, NODES * CH)
    v2 = np.ascontiguousarray(inputs["valid"], dtype=np.int32).reshape(
        S_TOT, NODES)
    in_maps = []
    for c in range(N_CORES):
        sl = slice(c * S, (c + 1) * S)
        in_maps.append({
            "h": h2[sl],
            "valid": v2[sl],
            "w1d": w1dv,
            "w2d": w2dv,
            "b1c": b1cv,
            "b2c": b2cv,
        })
    return in_maps


def kernel(h, valid, W1, b1, W2, b2):
    nc = _get_program()
    in_maps = _make_in_maps(
        {"h": h, "valid": valid, "W1": W1, "b1": b1, "W2": W2, "b2": b2})
    res = run_bass_kernel_spmd(nc, in_maps, core_ids=list(range(N_CORES)))
    outs = [res.results[c]["out"] for c in range(N_CORES)]
    full = np.concatenate(outs, axis=0).astype(np.float32)
    return full.reshape(S_TOT, NODES, 1)


# revision 13
# speedup vs baseline: 4.3921x; 1.0649x over previous
"""GroupedPNMLP forward on 8 Trainium2 NeuronCores (pure data parallel).

Per-node 2-layer MLP (32->32->1), 24 nodes in 6 groups of 4, with a
group-validity mask and node permutation.  Full inputs in, full output out;
samples are sharded N/8 per core, tiny weights replicated.

v4 pipeline (chunks of C=2048 samples, block-cyclic sample->partition):
  DMA h in two half-chunks (24KB contiguous per partition)
  -> ReLU + downcast to bf16 on DVE (2x_2p mode), written in a
     [n, t, c, s2] swizzle so neighbouring samples (s2) pair up
  -> DVE 32x32 block-transpose on the uint32 *pair* view: half the
     elements of an elementwise transpose; channels land on partitions
  -> mm1: ONE 128x128 block-diagonal bf16 matmul per node (4 copies of
     W1n on the diagonal -> all 4 sample sub-blocks in one shot); free
     dim 512 = one PSUM bank per node
  -> hidden drain: ReLU (+b1), PSUM->SBUF bf16 on ScalarE, one ACTIVATE
     per 2-bank pair
  -> mm2: 128x128 block-diagonal W2 (column slot = node id), 24 bf16
     matmuls accumulating into one PSUM bank; the rhs access pattern
     re-enumerates samples (hh,t,s2,pi) so pi is innermost again
  -> +b2 on ScalarE, DVE transpose back, group-valid mask multiply on
     GpSimd in natural layout, DMA out
  Mask path (small) runs on GpSimd + DVE off the critical path.
"""

import numpy as np
import ml_dtypes

import concourse.bass as bass
from concourse import bacc
import concourse.tile as tile
from concourse import mybir
from concourse.bass_utils import run_bass_kernel_spmd

F32 = mybir.dt.float32
BF16 = mybir.dt.bfloat16
I32 = mybir.dt.int32
U32 = mybir.dt.uint32

GROUPING = np.array(
    [[0, 3, 6, 9], [1, 4, 7, 10], [2, 5, 8, 11],
     [12, 13, 14, 15], [16, 18, 20, 22], [17, 19, 21, 23]], dtype=np.int32)

N_CORES = 8
S_TOT = 131072
S = S_TOT // N_CORES      # 16384 samples per core
NODES = 24
CH = 32                   # in channels = hidden dim
C = 2048                  # samples per chunk
NSUB = C // 128           # 16 samples per partition per chunk
HSUB = NSUB // 2          # 8 per half-chunk (t in 0..3, s2 in 0..1)
NCHUNK = S // C           # 8
NW = 6                    # waves of 4 nodes

# input-relu t-slices (of 4) on ScalarE; rest on DVE
ACT_T = 0


def _build_program():
    nc = bacc.Bacc(None, target_bir_lowering=False)

    h = nc.dram_tensor("h", [S, NODES * CH], F32, kind="ExternalInput")
    valid = nc.dram_tensor("valid", [S, NODES], I32, kind="ExternalInput")
    w1d = nc.dram_tensor("w1d", [128, NODES, 128], BF16, kind="ExternalInput")
    w2d = nc.dram_tensor("w2d", [128, NODES, 128], BF16, kind="ExternalInput")
    b1c = nc.dram_tensor("b1c", [128, NODES], F32, kind="ExternalInput")
    b2c = nc.dram_tensor("b2c", [128, 1], F32, kind="ExternalInput")
    out = nc.dram_tensor("out", [S, NODES], F32, kind="ExternalOutput")

    with tile.TileContext(nc) as tc:
        with (
            tc.tile_pool(name="singles", bufs=1) as singles,
            tc.tile_pool(name="xp", bufs=2) as xp,
            tc.tile_pool(name="xrp", bufs=2) as xrp,
            tc.tile_pool(name="xtp", bufs=2) as xtp,
            tc.tile_pool(name="hp", bufs=3) as hp,
            tc.tile_pool(name="vp", bufs=2) as vp,
            tc.tile_pool(name="op", bufs=2) as op,
            tc.tile_pool(name="php", bufs=3, space="PSUM") as php,
            tc.tile_pool(name="p2p", bufs=2, space="PSUM") as p2p,
        ):
            w1sb = singles.tile([128, NODES, 128], BF16)
            nc.sync.dma_start(out=w1sb, in_=w1d[:, :, :])
            w2sb = singles.tile([128, NODES, 128], BF16)
            nc.sync.dma_start(out=w2sb, in_=w2d[:, :, :])
            b1sb = singles.tile([128, NODES], F32)
            nc.sync.dma_start(out=b1sb, in_=b1c[:, :])
            b2sb = singles.tile([128, 1], F32)
            nc.sync.dma_start(out=b2sb, in_=b2c[:, :])

            for cc in range(NCHUNK):
                c0 = cc * C
                # ---- load h in halves, relu+cast bf16 swizzled, transpose ----
                # xt[32b+c, hh, n, t, 2*pi+s2] = relu(h[sample, n, c]) where
                # sample = c0 + hh*1024 + (32b+pi)*8 + 2*t + s2
                xt = xtp.tile([128, 2, NODES, 4, 64], BF16)
                for hh in range(2):
                    xh = xp.tile([128, HSUB, NODES, CH], F32)
                    lo = c0 + hh * (C // 2)
                    nc.sync.dma_start(
                        out=xh.rearrange("p s n c -> p (s n c)"),
                        in_=h[lo:lo + C // 2, :].rearrange(
                            "(p s) f -> p (s f)", p=128),
                    )
                    # xr[p, n, t, (c, s2)] = relu(xh[p, 2t+s2, n, c]) in bf16
                    # (3D-AP limit: one instruction per s2 phase)
                    xr = xrp.tile([128, NODES, 4, 64], BF16)
                    xr_v = xr.rearrange("p n t (c s2) -> p s2 t n c", s2=2)
                    xh_v = xh.rearrange("p (t s2) n c -> p s2 t n c", s2=2)
                    for s2 in range(2):
                        if ACT_T > 0:
                            nc.scalar.activation(
                                xr_v[:, s2, 0:ACT_T], xh_v[:, s2, 0:ACT_T],
                                mybir.ActivationFunctionType.Relu)
                        if ACT_T < 4:
                            nc.vector.tensor_scalar_max(
                                xr_v[:, s2, ACT_T:4], xh_v[:, s2, ACT_T:4],
                                0.0)
                    # u32 pair transpose: channels onto partitions
                    nc.vector.transpose(
                        xt[:, hh].bitcast(U32), xr.bitcast(U32))

                # ---- valid -> group mask (natural layout, off crit path) ----
                vi = vp.tile([128, NSUB, NODES], I32)
                for hh in range(2):
                    lo = c0 + hh * (C // 2)
                    nc.gpsimd.dma_start(
                        out=vi[:, hh * HSUB:(hh + 1) * HSUB],
                        in_=valid[lo:lo + C // 2, :].rearrange(
                            "(p s) n -> p s n", p=128),
                    )
                vf = vp.tile([128, NSUB, NODES], F32)
                nc.gpsimd.tensor_copy(vf, vi)
                gv = vp.tile([128, NSUB, 8], F32)
                nc.vector.tensor_reduce(
                    gv[:, :, 0:3],
                    vf[:, :, 0:12].rearrange("p s (k g) -> p s g k", g=3),
                    axis=mybir.AxisListType.X, op=mybir.AluOpType.add)
                nc.vector.tensor_reduce(
                    gv[:, :, 3:4], vf[:, :, 12:16],
                    axis=mybir.AxisListType.X, op=mybir.AluOpType.add)
                nc.vector.tensor_reduce(
                    gv[:, :, 4:6],
                    vf[:, :, 16:24].rearrange("p s (k g) -> p s g k", g=2),
                    axis=mybir.AxisListType.X, op=mybir.AluOpType.add)
                nc.vector.tensor_scalar(
                    gv[:, :, 0:6], gv[:, :, 0:6], 0.5, None,
                    op0=mybir.AluOpType.is_gt)
                mk = vp.tile([128, NSUB, NODES], F32)
                nc.gpsimd.tensor_copy(
                    mk[:, :, 0:12].rearrange("p s (k g) -> p s g k", g=3),
                    gv[:, :, 0:3].broadcast_to([128, NSUB, 3, 4]))
                nc.gpsimd.tensor_copy(
                    mk[:, :, 12:16],
                    gv[:, :, 3:4].broadcast_to([128, NSUB, 4]))
                nc.gpsimd.tensor_copy(
                    mk[:, :, 16:24].rearrange("p s (k g) -> p s g k", g=2),
                    gv[:, :, 4:6].broadcast_to([128, NSUB, 2, 4]))

                # ---- the 2-layer MLP on TensorE (block-diag 128x128) ----
                p2 = p2p.tile([128, NSUB, CH], F32)   # ((ht, s2), pi)
                p2f = p2.rearrange("p a b -> p (a b)")
                def emit_mm2(hidt, n0):
                    for q in range(2):
                        n = n0 + q
                        nc.tensor.matmul(
                            p2f,
                            lhsT=w2sb[:, n, :],
                            rhs=hidt[:, q].rearrange(
                                "p (ht pi s2) -> p ht s2 pi",
                                ht=8, s2=2),
                            start=(n == 0), stop=(n == NODES - 1),
                            skip_group_check=True)

                # software-pipelined: mm2 of half-wave k emits after the
                # mm1s of half-wave k+2, so drains never block the PE queue
                pend = []
                for k in range(2 * NW):
                    n0 = 2 * k
                    ph = php.tile([128, 2, 512], F32)
                    hidt = hp.tile([128, 2, 512], BF16)
                    for q in range(2):
                        nc.tensor.matmul(
                            ph[:, q, :],
                            lhsT=w1sb[:, n0 + q, :],
                            rhs=xt[:, :, n0 + q, :, :],
                            start=True, stop=True)
                    nc.scalar.activation(
                        hidt.rearrange("p a b -> p (a b)"),
                        ph.rearrange("p a b -> p (a b)"),
                        mybir.ActivationFunctionType.Relu,
                        bias=b1sb[:, n0:n0 + 1])
                    pend.append((hidt, n0))
                    if len(pend) > 2:
                        emit_mm2(*pend.pop(0))
                for args in pend:
                    emit_mm2(*args)

                # ---- +b2, transpose back, mask, store ----
                # p2 block f = (ht, s2) enumerates sub = hh*8 + 2t + s2
                # sequentially, so outT free is natural (sub, node) order.
                m2 = op.tile([128, NSUB, CH], F32)
                nc.scalar.activation(
                    m2.rearrange("p a b -> p (a b)"), p2f,
                    mybir.ActivationFunctionType.Identity,
                    bias=b2sb[:, 0:1])
                outT = op.tile([128, NSUB, CH], F32)
                nc.vector.transpose(outT, m2)
                outF = op.tile([128, NSUB, NODES], F32)
                nc.gpsimd.tensor_tensor(
                    outF, outT[:, :, 0:NODES], mk, op=mybir.AluOpType.mult)
                for hh in range(2):
                    lo = c0 + hh * (C // 2)
                    nc.gpsimd.dma_start(
                        out=out[lo:lo + C // 2, :].rearrange(
                            "(p s) n -> p s n", p=128),
                        in_=outF[:, hh * HSUB:(hh + 1) * HSUB],
                    )

    nc.compile()
    return nc


_PROGRAM = None


def _get_program():
    global _PROGRAM
    if _PROGRAM is None:
        _PROGRAM = _build_program()
    return _PROGRAM


def _prep_weights(W1, b1, W2, b2):
    flat = GROUPING.reshape(-1)
    g_of = np.zeros(NODES, np.int64)
    k_of = np.zeros(NODES, np.int64)
    for q, nid in enumerate(flat):
        g_of[nid] = q // 4
        k_of[nid] = q % 4
    W1n = np.ascontiguousarray(W1[g_of, k_of]).astype(np.float32)  # [24,32,32]
    W2n = np.ascontiguousarray(W2[g_of, k_of]).astype(np.float32)  # [24,32,1]
    b1n = np.ascontiguousarray(b1[g_of, k_of]).astype(np.float32)  # [24,32]
    b2n = np.ascontiguousarray(b2[g_of, k_of]).astype(np.float32)  # [24,1]

    w1dv = np.zeros((128, NODES, 128), np.float32)
    w2dv = np.zeros((128, NODES, 128), np.float32)
    b1cv = np.zeros((128, NODES), np.float32)
    b2cv = np.zeros((128, 1), np.float32)
    for n in range(NODES):
        for i in range(4):
            sl = slice(32 * i, 32 * i + 32)
            w1dv[sl, n, sl] = W1n[n]                       # [c, h] block
            w2dv[sl, n, 32 * i + n] = W2n[n][:, 0]         # column slot n
            b1cv[sl, n] = b1n[n]
            b2cv[32 * i + n, 0] = b2n[n, 0]
    return (w1dv.astype(ml_dtypes.bfloat16), w2dv.astype(ml_dtypes.bfloat16),
            b1cv, b2cv)


def _make_in_maps(inputs):
    w1dv, w2dv, b1cv, b2cv = _prep_weights(
        inputs["W1"], inputs["b1"], inputs["W2"], inputs["b2"])
    h2 = np.ascontiguousarray(inputs["h"], dtype=np.float32).reshape(
        S_TOT, NODES * CH)
    v2 = np.ascontiguousarray(inputs["valid"], dtype=np.int32).reshape(
        S_TOT, NODES)
    in_maps = []
    for c in range(N_CORES):
        sl = slice(c * S, (c + 1) * S)
        in_maps.append({
            "h": h2[sl],
            "valid": v2[sl],
            "w1d": w1dv,
            "w2d": w2dv,
            "b1c": b1cv,
            "b2c": b2cv,
        })
    return in_maps


def kernel(h, valid, W1, b1, W2, b2):
    nc = _get_program()
    in_maps = _make_in_maps(
        {"h": h, "valid": valid, "W1": W1, "b1": b1, "W2": W2, "b2": b2})
    res = run_bass_kernel_spmd(nc, in_maps, core_ids=list(range(N_CORES)))
    outs = [res.results[c]["out"] for c in range(N_CORES)]
    full = np.concatenate(outs, axis=0).astype(np.float32)
    return full.reshape(S_TOT, NODES, 1)
